# revision 48
# baseline (speedup 1.0000x reference)
"""AffinityPropagate Trainium2 kernel.

24 iterations of an 8-neighbor gated stencil:
    d <- (1-mask) * sum_k(gsh_k * shift_k(d)) / wsum + mask * blur

Strategy (8 NeuronCores, pure data parallel: one batch image per core):
  * Image [352, 1216] flattened row-major into SBUF [128 part x 3344].
  * Zero-padded 2D shifts == flat 1D shifted reads: per-direction gate
    weights are exactly 0 wherever a neighbor is out of bounds, so the
    row-wrap values the flat shift drags in are annihilated.
  * Per-direction weights W_k = |g_k shifted| * (1-mask)/wsum (fp16) are
    precomputed once; per iteration DVE does only the 8 fp16 multiplies
    (2x mode), PE sums the 8 product planes + b via identity-matmul PSUM
    accumulation, ACT casts PSUM->fp16 into ping-pong d tiles, and the
    +-1218-element halos are refreshed with SBUF->SBUF DMAs.
  * d_odd (d shifted by one element) keeps all shifted reads 4B-aligned
    so fp16 tensor_tensor stays in 2x mode for odd shift offsets.
"""

import numpy as np

from concourse import bass, mybir
from concourse.bass_utils import run_bass_kernel_spmd
from concourse.tile import TileContext

B, H, W = 8, 352, 1216
HW = H * W            # 428032
P = 128
F = HW // P           # 3344
HALO = 1218           # > max |shift| (1217), even
DW = HALO + F + HALO + 2  # d tile width (room for d_odd's +1 offset)
PROP_TIME = 24
NCHUNK_DVE = 4        # DVE multiply chunks per iteration
CD = F // NCHUNK_DVE  # 836
CP = 418              # one PSUM bank
OFFSETS = ((1, 1), (1, 0), (1, -1), (0, 1), (0, -1), (-1, 1), (-1, 0), (-1, -1))
SHIFTS = [dy * W + dx for dy, dx in OFFSETS]

f32 = mybir.dt.float32
f16 = mybir.dt.float16
MULT = mybir.AluOpType.mult
ADD = mybir.AluOpType.add

_CACHE = {}


def _split_sync_waits(nc, max_waits=1):
    """The walrus in this container accepts at most one sync-wait command
    per instruction; hoist extras onto preceding same-engine no-ops."""
    for f in nc.m.functions:
        for bb in f.blocks:
            out = []
            for inst in bb.instructions:
                si = inst.sync_info
                if si is not None and si.on_wait and len(si.on_wait) > max_waits:
                    waits = list(si.on_wait)
                    carry, keep = waits[:-max_waits], waits[-max_waits:]
                    for j, w in enumerate(carry):
                        out.append(mybir.InstNoOp(
                            name=f"{inst.name}-ws{j}", engine=inst.engine,
                            sync_info=mybir.SyncInfo(on_wait=[w], on_update=[]),
                            bass_nofuse=True))
                    inst.sync_info = mybir.SyncInfo(
                        on_wait=keep, on_update=list(si.on_update))
                out.append(inst)
            bb.instructions[:] = out


def _emit_shifted_plane_load(nc, gst, g, k, s, zrow):
    """gst[p, j] <- g[k, p*F + j + s], with every position whose 2D source
    is out of bounds forced to zero (rows here, wrap columns via masks)."""
    engs = (nc.sync, nc.scalar)
    eng = engs[k % 2]
    if s >= 0:
        for i, (p0, p1) in enumerate(((0, 32), (32, 64), (64, 96), (96, 127))):
            engs[(k + i) % 2].dma_start(
                out=gst[p0:p1, :],
                in_=g[k, s + p0 * F:s + p1 * F].rearrange(
                    "(p f) -> p f", p=p1 - p0))
        if s > 0:
            eng.dma_start(
                out=gst[127:128, 0:F - s],
                in_=g[k, s + 127 * F:HW].rearrange("(p f) -> p f", p=1))
        else:
            eng.dma_start(
                out=gst[127:128, :],
                in_=g[k, 127 * F:HW].rearrange("(p f) -> p f", p=1))
    else:
        a = -s
        eng.dma_start(
            out=gst[0:1, a:F],
            in_=g[k, 0:F - a].rearrange("(p f) -> p f", p=1))
        for i, (p0, p1) in enumerate(((1, 32), (32, 64), (64, 96), (96, 128))):
            engs[(k + i) % 2].dma_start(
                out=gst[p0:p1, :],
                in_=g[k, p0 * F - a:p1 * F - a].rearrange(
                    "(p f) -> p f", p=p1 - p0))
    dy = s // W if s >= 0 else -((-s + W - 1) // W)
    # top/bottom image rows (dy out of bounds) + DMA-uncovered slivers.
    # Partition-0 regions: DVE memset (legal start partition). Partition-127
    # regions: DMA from a zeros tile (compute APs must start on a quadrant).
    if s in (-1217, -1216, -1215):          # dy = -1
        nc.vector.memset(gst[0:1, 0:max(1216, -s)], 0.0)
    elif s == -1:
        nc.vector.memset(gst[0:1, 0:1], 0.0)
    elif s in (1215, 1216, 1217):           # dy = +1
        start = min(F - 1216, F - s)
        eng.dma_start(out=gst[127:128, start:F], in_=zrow[0:1, 0:F - start])
    elif s == 1:
        eng.dma_start(out=gst[127:128, F - 1:F], in_=zrow[0:1, 0:1])


def _halo_dmas(nc, d, dodd):
    """Refresh the flat-array halos of (d, d_odd) from d's own body."""
    # front halo of partition p = tail of partition p-1's body
    nc.sync.dma_start(out=d[1:128, 0:HALO], in_=d[0:127, F:F + HALO])
    nc.scalar.dma_start(out=dodd[1:128, 1:HALO + 1], in_=d[0:127, F:F + HALO])
    # back halo of partition p = head of partition p+1's body
    nc.sync.dma_start(out=d[0:127, HALO + F:HALO + F + HALO],
                      in_=d[1:128, HALO:2 * HALO])
    nc.scalar.dma_start(out=dodd[0:127, HALO + F + 1:HALO + F + HALO + 1],
                        in_=d[1:128, HALO:2 * HALO])


def _build():
    nc = bass.Bass()
    g = nc.dram_tensor("g", [8, HW], f32, kind="ExternalInput")
    blur = nc.dram_tensor("blur", [HW], f32, kind="ExternalInput")
    sparse = nc.dram_tensor("sparse", [HW], f32, kind="ExternalInput")
    maskL = nc.dram_tensor("maskL", [P, F], f16, kind="ExternalInput")
    maskR = nc.dram_tensor("maskR", [P, F], f16, kind="ExternalInput")
    ident = nc.dram_tensor("ident", [P, P], f16, kind="ExternalInput")
    out = nc.dram_tensor("out", [P, F], f32, kind="ExternalOutput")

    with TileContext(nc) as tc:
        with tc.tile_pool(name="const", bufs=1) as constp, \
             tc.tile_pool(name="wpool", bufs=1) as wpool, \
             tc.tile_pool(name="dpool", bufs=1) as dpool, \
             tc.tile_pool(name="misc", bufs=1) as miscp:

            identt = constp.tile([P, P], f16)
            nc.sync.dma_start(out=identt[:], in_=ident[:])
            zrow = constp.tile([P, 1220], f32)
            nc.vector.memset(zrow[:], 0.0)

            wt = [wpool.tile([P, F], f16, tag=f"w{k}", name=f"w{k}")
                  for k in range(8)]
            bt = miscp.tile([P, F], f16)

            dA = dpool.tile([P, DW], f16, tag="dA")
            dB = dpool.tile([P, DW], f16, tag="dB")
            doddA = dpool.tile([P, DW], f16, tag="doddA")
            doddB = dpool.tile([P, DW], f16, tag="doddB")
            for t in (dA, dB):
                nc.vector.memset(t[:, 0:HALO], 0.0)
                nc.vector.memset(t[:, HALO + F:DW], 0.0)
            for t in (doddA, doddB):
                # d_odd's body starts at HALO+1: cover j=HALO too (p0 keeps
                # reading it as the permanently-zero d_flat[-1])
                nc.vector.memset(t[:, 0:HALO + 1], 0.0)
                nc.vector.memset(t[:, HALO + F:DW], 0.0)

            # ---------------- preprocessing ----------------
            with tc.tile_pool(name="pre", bufs=2) as prep, \
                 tc.tile_pool(name="psumpre", bufs=4, space="PSUM") as psumpre:
                # guidance planes are the critical path: load them first
                maskLt = prep.tile([P, F], f16, tag="mL", bufs=1)
                maskRt = prep.tile([P, F], f16, tag="mR", bufs=1)
                nc.sync.dma_start(out=maskLt[:], in_=maskL[:])
                nc.scalar.dma_start(out=maskRt[:], in_=maskR[:])

                # wsum = sum of gate planes, accumulated on PE via identity
                # matmuls as each plane arrives (keeps DVE free)
                psw = [psumpre.tile([P, CP], f32, name=f"psw{q}", bufs=1,
                                    tag=f"psw{q}") for q in range(8)]
                # dx=0 planes last: their post-arrival chain skips the
                # wrap-column mask multiply
                for i, k in enumerate((0, 2, 3, 4, 5, 7, 1, 6)):
                    s = SHIFTS[k]
                    gst = prep.tile([P, F], f32, tag="gst", bufs=3)
                    _emit_shifted_plane_load(nc, gst, g, k, s, zrow)
                    # |g| -> fp16 gate plane
                    nc.scalar.activation(wt[k][:], gst[:],
                                         mybir.ActivationFunctionType.Abs)
                    dx = OFFSETS[k][1]
                    if dx == -1:
                        nc.vector.tensor_tensor(wt[k][:], wt[k][:], maskLt[:], MULT)
                    elif dx == 1:
                        nc.vector.tensor_tensor(wt[k][:], wt[k][:], maskRt[:], MULT)
                    for q in range(8):
                        qs = q * CP
                        nc.tensor.matmul(psw[q][:], identt[:],
                                         wt[k][:, qs:qs + CP],
                                         start=(i == 0), stop=(i == 7))

                # off the critical path: b / mask / d0
                sparse_st = prep.tile([P, F], f32, tag="gst", bufs=3)
                nc.sync.dma_start(
                    out=sparse_st[:], in_=sparse[:].rearrange("(p f) -> p f", p=P))
                blur_st = prep.tile([P, F], f32, tag="gst", bufs=3)
                nc.scalar.dma_start(
                    out=blur_st[:], in_=blur[:].rearrange("(p f) -> p f", p=P))
                m = prep.tile([P, F], f32, tag="m", bufs=1)
                nc.scalar.sign(m[:], sparse_st[:])
                nc.vector.tensor_tensor(bt[:], m[:], blur_st[:], MULT)  # b fp16
                # m <- 1 - m
                nc.vector.tensor_scalar(m[:], m[:], -1.0, 1.0, MULT, ADD)
                # d0 = blur (fp16 body + halos)
                nc.scalar.copy(out=dA[:, HALO:HALO + F], in_=blur_st[:])
                nc.scalar.copy(out=doddA[:, HALO + 1:HALO + 1 + F], in_=blur_st[:])
                _halo_dmas(nc, dA, doddA)

                # 1/wsum straight from the PSUM banks; chunked so iteration 1
                # can start on chunk 0 early
                winv = prep.tile([P, F], f32, tag="winv", bufs=1)
                winvh = prep.tile([P, F], f16, tag="mR", bufs=1)
                for c in range(NCHUNK_DVE):
                    sl = slice(c * CD, (c + 1) * CD)
                    for q in range(c * CD // CP, (c + 1) * CD // CP):
                        qs = q * CP
                        nc.vector.reciprocal(winv[:, qs:qs + CP], psw[q][:])
                    nc.vector.tensor_tensor(winv[:, sl], winv[:, sl],
                                            m[:, sl], MULT)
                    nc.scalar.copy(out=winvh[:, sl], in_=winv[:, sl])
                    # W_k = gate_k * (1-mask)/wsum   (fp16, in place, 2x mode)
                    for k in range(8):
                        nc.vector.tensor_tensor(wt[k][:, sl], wt[k][:, sl],
                                                winvh[:, sl], MULT)

            # ---------------- 24 stencil iterations ----------------
            with tc.tile_pool(name="prod", bufs=2) as prodp, \
                 tc.tile_pool(name="psum", bufs=4, space="PSUM") as psump, \
                 tc.tile_pool(name="post", bufs=1) as postp:

                src = (dA, doddA)
                dst = (dB, doddB)
                NSUB = CD // CP
                # chunks whose body feeds each halo side
                back_set = {c for c in range(NCHUNK_DVE) if c * CD < HALO}
                front_set = {c for c in range(NCHUNK_DVE)
                             if (c + 1) * CD > F - HALO}
                ostage = postp.tile([P, F], f32)
                for it in range(PROP_TIME):
                    last = it == PROP_TIME - 1
                    d_s, dodd_s = src
                    d_d, dodd_d = dst
                    order = range(NCHUNK_DVE) if it % 2 == 0 \
                        else range(NCHUNK_DVE - 1, -1, -1)
                    back_done = front_done = False
                    done = set()
                    for c in order:
                        cs = c * CD
                        prods = []
                        for k, s in enumerate(SHIFTS):
                            if s % 2 == 0:
                                base = HALO + s
                                srct = d_s
                            else:
                                base = HALO + 1 + s
                                srct = dodd_s
                            pr = prodp.tile([P, CD], f16, tag=f"pr{k}")
                            nc.vector.tensor_tensor(
                                pr[:], wt[k][:, cs:cs + CD],
                                srct[:, base + cs:base + cs + CD], MULT)
                            prods.append(pr)
                        for h in range(NSUB):
                            hs = h * CP
                            ps = psump.tile([P, CP], f32)
                            nc.tensor.matmul(ps[:], identt[:],
                                             bt[:, cs + hs:cs + hs + CP],
                                             start=True, stop=False)
                            for k in range(8):
                                nc.tensor.matmul(ps[:], identt[:],
                                                 prods[k][:, hs:hs + CP],
                                                 start=False, stop=(k == 7))
                            if last:
                                # stream final result straight to DRAM (fp32)
                                nc.scalar.copy(
                                    out=ostage[:, cs + hs:cs + hs + CP],
                                    in_=ps[:])
                                nc.sync.dma_start(
                                    out=out[:, cs + hs:cs + hs + CP],
                                    in_=ostage[:, cs + hs:cs + hs + CP])
                            else:
                                nc.scalar.copy(
                                    out=d_d[:, HALO + cs + hs:HALO + cs + hs + CP],
                                    in_=ps[:])
                                nc.scalar.copy(
                                    out=dodd_d[:, HALO + 1 + cs + hs:HALO + 1 + cs + hs + CP],
                                    in_=ps[:])
                        if last:
                            continue
                        done.add(c)
                        # launch halo refreshes as soon as their source body
                        # chunks have been written
                        if not back_done and back_set <= done:
                            nc.sync.dma_start(
                                out=d_d[0:127, HALO + F:HALO + F + HALO],
                                in_=d_d[1:128, HALO:2 * HALO])
                            nc.scalar.dma_start(
                                out=dodd_d[0:127, HALO + F + 1:HALO + F + HALO + 1],
                                in_=d_d[1:128, HALO:2 * HALO])
                            back_done = True
                        if not front_done and front_set <= done:
                            nc.sync.dma_start(
                                out=d_d[1:128, 0:HALO],
                                in_=d_d[0:127, F:F + HALO])
                            nc.scalar.dma_start(
                                out=dodd_d[1:128, 1:HALO + 1],
                                in_=d_d[0:127, F:F + HALO])
                            front_done = True
                    src, dst = dst, src

    nc.finalize()
    _split_sync_waits(nc)
    return nc


def _consts():
    j = np.arange(HW, dtype=np.int64) % W
    mL = (j != 0).astype(np.float16).reshape(P, F)
    mR = (j != W - 1).astype(np.float16).reshape(P, F)
    return mL, mR, np.eye(P, dtype=np.float16)


def kernel(guidance, blur_depth, sparse_depth):
    if "nc" not in _CACHE:
        _CACHE["nc"] = _build()
    nc = _CACHE["nc"]
    guidance = np.asarray(guidance, dtype=np.float32)
    blur_depth = np.asarray(blur_depth, dtype=np.float32)
    sparse_depth = np.asarray(sparse_depth, dtype=np.float32)
    mL, mR, idm = _consts()
    in_maps = []
    for c in range(B):
        in_maps.append({
            "g": np.ascontiguousarray(guidance[c].reshape(8, HW)),
            "blur": np.ascontiguousarray(blur_depth[c].reshape(HW)),
            "sparse": np.ascontiguousarray(sparse_depth[c].reshape(HW)),
            "maskL": mL, "maskR": mR, "ident": idm,
        })
    # every iterate is a convex combination of blur_depth values, so the
    # output must stay inside blur's range; violations mean the device
    # glitched (transient NRT wedge) -> retry
    lo = float(blur_depth.min()) - 1e-2
    hi = float(blur_depth.max()) + 1e-2

    import time
    outp = None
    for attempt in range(4):
        try:
            res = run_bass_kernel_spmd(nc, in_maps, list(range(B)))
            outp = np.stack(
                [res.results[c]["out"].reshape(1, H, W) for c in range(B)])
            if np.isfinite(outp).all() and outp.min() >= lo and outp.max() <= hi:
                return outp
            print(f"kernel: attempt {attempt} produced out-of-range values; "
                  f"retrying", flush=True)
        except Exception as e:
            # transient NRT device-unrecoverable states clear on a retry
            if attempt == 3:
                raise
            print(f"kernel: attempt {attempt} failed ({type(e).__name__}); "
                  f"retrying", flush=True)
        time.sleep(20 * (attempt + 1))
    return outp

